# revision 1
# baseline (speedup 1.0000x reference)
"""Trainium2 Bass kernel for nn_CustomLoss: weighted-CE + all-pairs windowed SSIM BCE loss.

Strategy: pure data-parallel over batch B=32 -> 4 videos per core on 8 cores.
Math is done on raw (unnormalized) 7x7 window sums; the /49 window norms and
the 49/48 covariance factor fold into band-matrix scales and scalar constants
(SSIM is scale-invariant in num/den), so no per-element rescaling is needed.

Per core, per video (layout: partitions = H(64) x channel-parity q(2) = 128,
free axis = [F, CP=8, W]):
  - DMA bf16 features (converted on host; halves HBM traffic)
  - X2 = x^2 on ScalarE (Square); 2-tap W pre-sums of x and x^2 on DVE
  - per-frame U = 49*ux, Q = 2401*uxx via 4-tap banded matmuls on TensorE
    (taps over pre-summed pairs), PSUM evacuated by ScalarE
  - per-pair P = 2401*uxy via 7-tap banded matmuls (band carries the 49x)
  - SSIM map algebra split across DVE (muls/subs, tensor_scalar at 4x mode),
    Pool (den1/den2 adds), ScalarE (rsqrt; square folded into two DVE muls)
  - per-pair spatial sums via 1-column ones-matmuls into distinct partitions
    of one PSUM bank; single ScalarE evacuation + DMA out per video
Host: tiny tail (28 pair sums -> ssim means -> BCE; CE from predictions).
"""

import numpy as np
import ml_dtypes

B, F, C, H, W = 32, 8, 16, 64, 64
NCORES = 8
BSH = B // NCORES          # 4 videos per core
CP = C // 2                # channel pairs stacked on partitions
WIN = 7
HO = H - WIN + 1           # 58
NP_WIN = WIN * WIN
COV_NORM = NP_WIN / (NP_WIN - 1.0)
NPAIR = F * (F - 1) // 2   # 28
NPART = 2 * HO             # 116 used partitions

# constants in raw-sum space (everything scaled by 49^2 = 2401)
C1P = 2401.0 * (0.01 ** 2)          # 0.2401
C2P = 2401.0 * (0.03 ** 2)          # 2.1609
TWO_COV = 2.0 * COV_NORM

_CACHE = {}


def _pair_index(i, j):
    # triu order (row-major), matches np.triu_indices(F, 1)
    base = i * (2 * F - i - 1) // 2
    return base + (j - i - 1)


def _build_program():
    import concourse.bass as bass
    import concourse.bacc as bacc
    import concourse.tile as tile
    from concourse import mybir

    f32 = mybir.dt.float32
    bf16 = mybir.dt.bfloat16
    AF = mybir.ActivationFunctionType

    nc = bacc.Bacc(None, target_bir_lowering=False)

    feat = nc.dram_tensor([BSH, F, C, H, W], bf16, kind="ExternalInput")
    # bands: cols 0:116 -> weight 1 block-diag, cols 116:232 -> weight 49
    band = nc.dram_tensor([128, 2 * NPART], bf16, kind="ExternalInput")
    # reduce selector: single ones-column at index NPAIR-1; slicing cols
    # [NPAIR-1-p : 2*NPAIR-1-p) puts the ones-column at slice position p
    redsel = nc.dram_tensor([128, 2 * NPAIR], bf16, kind="ExternalInput")
    negid = nc.dram_tensor([128, NPART], bf16, kind="ExternalInput")
    out = nc.dram_tensor([BSH, NPAIR, CP * HO], bf16, kind="ExternalOutput")

    # element strides of feat
    s_b = F * C * H * W
    s_f = C * H * W
    s_c = H * W

    def ap_of(x):
        return x[:] if not isinstance(x, bass.AP) else x

    with tile.TileContext(nc) as tc:
        with (
            tc.tile_pool(name="consts", bufs=1) as consts,
            tc.tile_pool(name="stage", bufs=2) as stage_p,
            tc.tile_pool(name="xp", bufs=2) as xp,
            tc.tile_pool(name="frameq", bufs=2) as frameq,
            tc.tile_pool(name="transq", bufs=1) as transq,
            tc.tile_pool(name="pairp", bufs=2) as pairp,
            tc.tile_pool(name="math", bufs=3) as mathp,
            tc.tile_pool(name="denp", bufs=3) as denp,
            tc.tile_pool(name="rsqp", bufs=2) as rsqp,
            tc.tile_pool(name="psum_f", bufs=3, space="PSUM") as psum_f,
            tc.tile_pool(name="psum_p", bufs=3, space="PSUM") as psum_p,
            tc.tile_pool(name="psum_red", bufs=2, space="PSUM") as psum_red,
            tc.tile_pool(name="obuf_p", bufs=1) as obuf_p,
        ):
            band_sb = consts.tile([128, 2 * NPART], bf16)
            nc.sync.dma_start(out=band_sb[:], in_=band[:])
            redsel_sb = consts.tile([128, 2 * NPAIR], bf16)
            nc.sync.dma_start(out=redsel_sb[:], in_=redsel[:])
            negid_sb = consts.tile([128, NPART], bf16)
            nc.sync.dma_start(out=negid_sb[:], in_=negid[:])
            band1 = band_sb[:, 0:NPART]
            band49 = band_sb[:, NPART:2 * NPART]

            def bcast_j(t, i, nj):
                # t is a tile [p, F, CP, X]; return AP [p, nj, CP, X] broadcasting f=i
                base = t[:, i, :, :]
                return bass.AP(
                    tensor=base.tensor,
                    offset=base.offset,
                    ap=[base.ap[0], [0, nj]] + list(base.ap[1:]),
                )

            def emit_stage(b):
                stg = stage_p.tile([128, F, CP, W], bf16, tag="stg")
                for q in range(2):
                    src = ap_of(feat)
                    src_ap = bass.AP(
                        tensor=src.tensor,
                        offset=src.offset + b * s_b + q * s_c,
                        ap=[[W, H], [s_f, F], [2 * s_c, CP], [1, W]],
                    )
                    nc.sync.dma_start(out=stg[64 * q:64 * q + 64, :, :, :], in_=src_ap)
                return stg

            def emit_presums(stg):
                # 2-tap W pre-sums (a[w] = x[w] + x[w+1], w in [0,63))
                ax = xp.tile([128, F, CP, W], bf16, tag="ax")
                nc.vector.tensor_add(
                    ax[:, :, :, 0:W - 1], stg[:, :, :, 0:W - 1], stg[:, :, :, 1:W]
                )
                X2 = xp.tile([128, F, CP, W], bf16, tag="X2")
                nc.scalar.activation(X2[:], stg[:], AF.Square)
                axx = xp.tile([128, F, CP, W], bf16, tag="axx")
                nc.vector.tensor_add(
                    axx[:, :, :, 0:W - 1], X2[:, :, :, 0:W - 1], X2[:, :, :, 1:W]
                )
                return X2, ax, axx

            def emit_filters(stg, pre):
                X2, ax, axx = pre
                # per-frame window sums: U = 49*ux (band 1), Q = 2401*uxx (band 49)
                U = frameq.tile([128, F, CP, HO], bf16, tag="U")
                Q = transq.tile([128, F, CP, HO], bf16, tag="Q")
                for kf in range(F):
                    for bnd, prt, full, dst in (
                        (band1, ax, stg, U),
                        (band49, axx, X2, Q),
                    ):
                        ps = psum_f.tile([128, CP * HO], f32, tag="psf")
                        for ti, (srt, dw) in enumerate(
                            ((prt, 0), (prt, 2), (prt, 4), (full, 6))
                        ):
                            nc.tensor.matmul(
                                ps[0:NPART, :],
                                bnd,
                                srt[:, kf, :, dw:dw + HO],
                                start=(ti == 0),
                                stop=(ti == 3),
                            )
                        nc.scalar.activation(
                            dst[0:NPART, kf, :, :], ps[0:NPART, :], AF.Copy
                        )

                # per-frame algebra (in place):
                #   A1 = U^2 + C1'/2 ; V1 = C'(Q - U^2) + C2'/2
                A1 = frameq.tile([128, F, CP, HO], bf16, tag="A1")
                nc.scalar.activation(A1[0:NPART], U[0:NPART], AF.Square)
                V1 = frameq.tile([128, F, CP, HO], bf16, tag="V1")
                nc.vector.tensor_sub(V1[0:NPART], Q[0:NPART], A1[0:NPART])
                nc.vector.tensor_scalar(
                    V1[0:NPART], V1[0:NPART], COV_NORM, C2P / 2.0,
                    mybir.AluOpType.mult, mybir.AluOpType.add,
                )
                nc.vector.tensor_scalar_add(A1[0:NPART], A1[0:NPART], C1P / 2.0)
                return U, A1, V1

            def emit_reduce(red, i, jl, jn, m):
                for j in range(jn):
                    p = _pair_index(i, jl + j)
                    nc.tensor.matmul(
                        red[0:NPAIR, :],
                        redsel_sb[0:NPART, NPAIR - 1 - p:2 * NPAIR - 1 - p],
                        m[0:NPART, j, :, :],
                        start=(p == 0),
                        stop=(p == NPAIR - 1),
                    )

            DEFER = 2
            stg = emit_stage(0)
            pre = emit_presums(stg)
            U, A1, V1 = emit_filters(stg, pre)
            nxt_stg = nxt_pre = None
            for b in range(BSH):
                red = psum_red.tile([128, CP * HO], f32, tag="red")

                # sub-batches of <= NJB pairs; den-chain (Pool/DVE/ACT,
                # depends only on per-frame A1/V1) is emitted ahead of the
                # uxy-chain so it pipelines past the PE taps
                NJB = 4
                batches = []
                for i in range(F - 1):
                    nj = F - 1 - i
                    for j0 in range(0, nj, NJB):
                        batches.append((i, j0, min(NJB, nj - j0)))

                tcur = None
                deferred = []
                for bi, (i, j0, jn) in enumerate(batches):
                    if bi == 2 and b + 1 < BSH:
                        # software pipeline: stage + presums for next video
                        nxt_stg = emit_stage(b + 1)
                        nxt_pre = emit_presums(nxt_stg)
                    if j0 == 0:
                        nj = F - 1 - i
                        tcur = pairp.tile([128, nj, CP, W], bf16, tag="t")
                        nc.vector.tensor_mul(
                            tcur[:], bcast_j(stg, i, nj), stg[:, i + 1:F, :, :]
                        )
                        tpre = pairp.tile([128, nj, CP, W], bf16, tag="tpre")
                        nc.vector.tensor_add(
                            tpre[:, :, :, 0:W - 1], tcur[:, :, :, 0:W - 1],
                            tcur[:, :, :, 1:W]
                        )
                    jl, jh = i + 1 + j0, i + 1 + j0 + jn

                    den = denp.tile([128, jn, CP, HO], bf16, tag="den1")
                    nc.gpsimd.tensor_add(
                        den[0:NPART, 0:jn], bcast_j(A1, i, jn)[0:NPART],
                        A1[0:NPART, jl:jh, :, :]
                    )
                    den2 = denp.tile([128, jn, CP, HO], bf16, tag="den2")
                    nc.gpsimd.tensor_add(
                        den2[0:NPART, 0:jn], bcast_j(V1, i, jn)[0:NPART],
                        V1[0:NPART, jl:jh, :, :]
                    )
                    nc.vector.tensor_mul(den[0:NPART], den[0:NPART], den2[0:NPART])
                    rsq = rsqp.tile([128, jn, CP, HO], bf16, tag="rsq")
                    nc.scalar.activation(rsq[0:NPART], den[0:NPART], AF.Abs_reciprocal_sqrt)
                    r2 = rsqp.tile([128, jn, CP, HO], bf16, tag="r2")
                    nc.scalar.activation(r2[0:NPART], rsq[0:NPART], AF.Square)

                    m = mathp.tile([128, jn, CP, HO], bf16, tag="m")
                    nc.vector.tensor_mul(
                        m[0:NPART], bcast_j(U, i, jn)[0:NPART],
                        U[0:NPART, jl:jh, :, :]
                    )
                    # PSUM accumulates P - m (f32); evac applies 2C'x + C2'
                    # so uxy holds num2 directly
                    uxy = pairp.tile([128, jn, CP, HO], bf16, tag="uxy")
                    for j in range(j0, j0 + jn):
                        ps = psum_p.tile([128, CP * HO], f32, tag="psp")
                        nc.tensor.matmul(
                            ps[0:NPART, :], negid_sb[0:NPART, :],
                            m[0:NPART, j - j0, :, :], start=True, stop=False,
                        )
                        for ti, (srt, dw) in enumerate(
                            ((tpre, 0), (tpre, 2), (tpre, 4), (tcur, 6))
                        ):
                            nc.tensor.matmul(
                                ps[0:NPART, :],
                                band49,
                                srt[:, j, :, dw:dw + HO],
                                start=False,
                                stop=(ti == 3),
                            )
                        nc.scalar.activation(
                            uxy[0:NPART, j - j0, :, :], ps[0:NPART, :], AF.Copy,
                            scale=TWO_COV, bias=C2P,
                        )

                    # in-place chain: m -> num1 -> num -> S
                    nc.vector.tensor_scalar(
                        m[0:NPART], m[0:NPART], 2.0, C1P,
                        mybir.AluOpType.mult, mybir.AluOpType.add,
                    )
                    nc.vector.tensor_mul(m[0:NPART], m[0:NPART], uxy[0:NPART])
                    nc.vector.tensor_mul(m[0:NPART], m[0:NPART], r2[0:NPART])

                    # spatial sums: pair p -> partition p of the shared psum
                    # bank (selector slice has its ones-column at position p;
                    # other partitions accumulate zeros). The last DEFER
                    # batches are emitted after the next video's filter
                    # matmuls so PE is not serialized behind the DVE tail.
                    deferred.append((i, jl, jn, m))
                    if bi < len(batches) - DEFER:
                        emit_reduce(red, *deferred.pop(0))

                if b + 1 < BSH:
                    stg, pre = nxt_stg, nxt_pre
                    U, A1, V1 = emit_filters(stg, pre)
                for args in deferred:
                    emit_reduce(red, *args)
                gbuf = obuf_p.tile([128, CP * HO], bf16, tag="gbuf")
                nc.scalar.activation(gbuf[0:NPAIR, :], red[0:NPAIR, :], AF.Copy)
                nc.sync.dma_start(out=out[b, :, :], in_=gbuf[0:NPAIR, :])

    nc.compile()
    return nc, feat.name, band.name, redsel.name, negid.name, out.name


def _make_consts():
    band = np.zeros((128, 2 * NPART), dtype=np.float32)
    for scale_i, scale in enumerate((1.0, 49.0)):
        for s in range(2):
            for ho in range(HO):
                band[64 * s + ho:64 * s + ho + WIN,
                     scale_i * NPART + HO * s + ho] = scale
    redsel = np.zeros((128, 2 * NPAIR), dtype=np.float32)
    redsel[0:NPART, NPAIR - 1] = 1.0
    negid = np.zeros((128, NPART), dtype=np.float32)
    negid[0:NPART, 0:NPART] = -np.eye(NPART)
    return (band.astype(ml_dtypes.bfloat16), redsel.astype(ml_dtypes.bfloat16),
            negid.astype(ml_dtypes.bfloat16))


def kernel(predictions, features, labels):
    from concourse.bass_utils import run_bass_kernel_spmd

    if "prog" not in _CACHE:
        _CACHE["prog"] = _build_program()
    nc, feat_name, band_name, redsel_name, negid_name, out_name = _CACHE["prog"]

    band, redsel, negid = _make_consts()
    feats = np.asarray(features, dtype=np.float32).astype(ml_dtypes.bfloat16)
    in_maps = [
        {
            feat_name: np.ascontiguousarray(feats[k * BSH:(k + 1) * BSH]),
            band_name: band,
            redsel_name: redsel,
            negid_name: negid,
        }
        for k in range(NCORES)
    ]
    res = run_bass_kernel_spmd(nc, in_maps, core_ids=list(range(NCORES)))
    sums = np.concatenate([r[out_name] for r in res.results], axis=0)  # [32, 28, CP*HO]

    # S is already unscaled (num and den both carry the 49^2 factor)
    ssim_pair = sums.astype(np.float64).sum(-1) / (C * HO * HO)  # [32, 28]

    labels = np.asarray(labels).astype(np.int64)
    preds = np.asarray(predictions).astype(np.float64)

    # weighted CE (torch CrossEntropyLoss with weights [10, 1])
    mx = preds.max(axis=1, keepdims=True)
    logp = preds - mx - np.log(np.exp(preds - mx).sum(axis=1, keepdims=True))
    nll = -logp[np.arange(B), labels]
    wts = np.where(labels == 0, 10.0, 1.0)
    cce = (wts * nll).sum() / wts.sum()

    # BCE on mean pair-similarity
    sim = np.clip(ssim_pair + 0.5, 0.0, 1.0)
    avg_sim = sim.mean(axis=1)
    t = (labels == 0).astype(np.float64)
    log_p = np.maximum(np.log(np.maximum(avg_sim, 1e-300)), -100.0)
    log_1mp = np.maximum(np.log(np.maximum(1.0 - avg_sim, 1e-300)), -100.0)
    bce = -(t * log_p + (1.0 - t) * log_1mp)
    inconsistency = bce.mean()

    return np.float32(cce + 4.0 * inconsistency)

